# revision 11
# baseline (speedup 1.0000x reference)
"""BotSpot GNN message-passing kernel for 8 Trainium2 NeuronCores.

Strategy (per sharding hint): shard the 200k device nodes (and adjacency
rows) across the 8 cores; each core computes a partial per-channel
neighbor sum with the tensor engine (adj streamed as fp8, dev embeddings
as fp16 weights), all-reduce the [114, 1000] partials, replicate the
small channel tables / MLP params, and data-parallel split the edge
batch by the owner core of each edge's device node.

Embedding lookups run on-device via the gpsimd ap_gather extended
instruction with embedding tables stored feature-major (table k's 16
feature rows live on partitions 16k..16k+15, so gpsimd core k serves
table k with its own per-core index stream).

Layout note: gather position j maps to shard-local device d = (j % 128)
* 196 + j // 128, so the adjacency rows a tensor-engine chunk needs are
per-partition contiguous in HBM (full-rate DMA); all index arrays are
pre-permuted accordingly on the host.
"""
import sys
import os
import numpy as np

for _p in ("/opt/trn_rl_repo", os.path.expanduser("~/.axon_site/_ro/trn_rl_repo")):
    if os.path.isdir(_p) and _p not in sys.path:
        sys.path.insert(0, _p)

import concourse.bass as bass
import concourse.tile as tile
from concourse import bacc, mybir, library_config
from concourse.bass_utils import run_bass_kernel_spmd
from concourse.tile import add_dep_helper

F32 = mybir.dt.float32
F16 = mybir.dt.float16
FP8 = mybir.dt.float8e4
U8 = mybir.dt.uint8
I16 = mybir.dt.int16

N_CORES = 8
N_DEV = 200_000
N_CHAN = 1000
B = 65_536
CARDS = [40, 4, 100, 200, 300, 500, 150]
NT = 7                      # categorical tables
NS = 25_088                 # padded devices per core (196 x 128)
NCHUNK = NS // 128          # 196
SLAB = 1792                 # devices per slab (14 chunks)
NSLAB = NS // SLAB          # 14
CPS = SLAB // 128           # 14 chunks per slab
D_FEAT = 113                # num + 7*16
D_AUG = 114                 # + ones row (for deg)
NB = 10240                  # padded batch per core (5 x 2048)
BBLK = 2048                 # batch block columns
MAXC = 512                  # emb table entries per partition (>= max card)

D1, D2P = 67, 64            # dev tower dims (h2 padded 50->64)
C1 = 14
FUS_OUT = 48
CC1, CC2 = 58, 29

TRACE = False               # set by test harness for profiling
TRACE_DIR = None            # optional dir to keep NTFF artifacts

_CACHE = {}


def _build_nc():
    nc = bacc.Bacc("TRN2", target_bir_lowering=False, debug=False,
                   num_devices=N_CORES)
    dt = nc.dram_tensor
    # per-core sharded inputs
    adj_s = dt("adj_s", [NS, N_CHAN], U8, kind="ExternalInput")
    num_ones = dt("num_ones", [2, NS], F32, kind="ExternalInput")
    cat_idx = dt("cat_idx", [NT * 16, NS // 16], I16, kind="ExternalInput")
    dev_idx_w = dt("dev_idx_w", [64, NB // 16], I16, kind="ExternalInput")
    chan_idx_w = dt("chan_idx_w", [64, NB // 16], I16, kind="ExternalInput")
    # replicated tables / params
    emb_tbl = dt("emb_tbl", [NT * 16, MAXC], F32, kind="ExternalInput")
    ident = dt("ident", [D_AUG, D_AUG], F32, kind="ExternalInput")
    w1 = dt("w1", [D_FEAT, D1], F16, kind="ExternalInput")
    b1 = dt("b1", [D1, 1], F32, kind="ExternalInput")
    w2 = dt("w2", [D1, D2P], F16, kind="ExternalInput")
    b2 = dt("b2", [D2P, 1], F32, kind="ExternalInput")
    msg_w = dt("msg_w", [D_FEAT, D1], F32, kind="ExternalInput")
    msg_b = dt("msg_b", [D1, 1], F32, kind="ExternalInput")
    chan_w_a = dt("chan_w_a", [8, C1], F32, kind="ExternalInput")
    chan_w_b = dt("chan_w_b", [16, C1], F32, kind="ExternalInput")
    chan_b = dt("chan_b", [C1, 1], F32, kind="ExternalInput")
    chan_num_t = dt("chan_num_t", [8, N_CHAN], F32, kind="ExternalInput")
    chan_id_tbl = dt("chan_id_tbl", [16, N_CHAN], F32, kind="ExternalInput")
    chan_id_w = dt("chan_id_w", [16, 64], I16, kind="ExternalInput")
    fus_wa = dt("fus_wa", [C1, FUS_OUT], F32, kind="ExternalInput")
    fus_wb = dt("fus_wb", [D1, FUS_OUT], F32, kind="ExternalInput")
    fus_b = dt("fus_b", [FUS_OUT, 1], F32, kind="ExternalInput")
    ccw1_top = dt("ccw1_top", [FUS_OUT, 64], F32, kind="ExternalInput")
    ccw1_bot = dt("ccw1_bot", [50, CC1], F16, kind="ExternalInput")
    ccb1 = dt("ccb1", [CC1, 1], F32, kind="ExternalInput")
    ccw2 = dt("ccw2", [CC1, CC2], F16, kind="ExternalInput")
    ccb2 = dt("ccb2", [CC2, 1], F32, kind="ExternalInput")
    ccw3 = dt("ccw3", [CC2, 1], F16, kind="ExternalInput")
    ccb3 = dt("ccb3", [1, 1], F32, kind="ExternalInput")
    out_logits = dt("out_logits", [1, NB], F32, kind="ExternalOutput")

    # partition p holds shard rows [p*196, (p+1)*196) -> contiguous loads
    adj_view = adj_s[:].rearrange("(p g) e -> p g e", p=128)

    Relu = mybir.ActivationFunctionType.Relu

    with tile.TileContext(nc) as tc:
        with tc.tile_pool(name="const", bufs=1) as const, \
             tc.tile_pool(name="big", bufs=1) as big, \
             tc.tile_pool(name="dram", bufs=1, space="DRAM") as dram:
            # ---- load constants ----
            def load(src, shape, dtype=F32):
                t = const.tile(shape, dtype, tag=src.name + "_t")
                nc.sync.dma_start(out=t[:], in_=src[:])
                return t

            tbl_t = load(emb_tbl, [NT * 16, MAXC])
            id_t = load(ident, [D_AUG, D_AUG])
            cat_t = load(cat_idx, [NT * 16, NS // 16], I16)
            w1_t = load(w1, [D_FEAT, D1], F16); b1_t = load(b1, [D1, 1])
            w2_t = load(w2, [D1, D2P], F16); b2_t = load(b2, [D2P, 1])
            msgw_t = load(msg_w, [D_FEAT, D1]); msgb_t = load(msg_b, [D1, 1])
            cwa_t = load(chan_w_a, [8, C1]); cwb_t = load(chan_w_b, [16, C1])
            cb_t = load(chan_b, [C1, 1])
            cnum_t = load(chan_num_t, [8, N_CHAN])
            cid_tbl_t = load(chan_id_tbl, [16, N_CHAN])
            cid_w_t = load(chan_id_w, [16, 64], I16)
            fwa_t = load(fus_wa, [C1, FUS_OUT]); fwb_t = load(fus_wb, [D1, FUS_OUT])
            fb_t = load(fus_b, [FUS_OUT, 1])
            c1top_t = load(ccw1_top, [FUS_OUT, 64])
            c1bot_t = load(ccw1_bot, [50, CC1], F16); cb1_t = load(ccb1, [CC1, 1])
            c2_t = load(ccw2, [CC1, CC2], F16); cb2_t = load(ccb2, [CC2, 1])
            c3_t = load(ccw3, [CC2, 1], F16); cb3_t = load(ccb3, [1, 1])
            didx_t = load(dev_idx_w, [64, NB // 16], I16)
            cidx_t = load(chan_idx_w, [64, NB // 16], I16)

            ll = nc.gpsimd.load_library(library_config.ap_gather)
            gathers = []

            h2full = big.tile([D2P, NS], F32)       # device tower output
            ch2full = big.tile([64, N_CHAN], F32)   # channel head table
            nbr_sb = big.tile([D_AUG, N_CHAN], F32)

            # ---------------- phase A: shard aggregation + tower ----------
            with tc.tile_pool(name="slab", bufs=2) as slabp, \
                 tc.tile_pool(name="adjp", bufs=3) as adjp, \
                 tc.tile_pool(name="dvp", bufs=16) as dvp, \
                 tc.tile_pool(name="h1p", bufs=2) as h1p, \
                 tc.tile_pool(name="psA", bufs=1, space="PSUM") as psA, \
                 tc.tile_pool(name="psT", bufs=4, space="PSUM") as psT, \
                 tc.tile_pool(name="psH", bufs=1, space="PSUM") as psH:

                ps_nbr0 = psA.tile([D_AUG, 500], F32, tag="nbr0")
                ps_nbr1 = psA.tile([D_AUG, 500], F32, tag="nbr1")

                for s in range(NSLAB):
                    emb_slab = slabp.tile([D_AUG, SLAB], F32, tag="emb_slab")
                    g = nc.gpsimd.ap_gather(
                        out_ap=emb_slab[0:112, :], in_ap=tbl_t[:],
                        idxs_ap=cat_t[:, s * (SLAB // 16):(s + 1) * (SLAB // 16)],
                        channels=112, num_elems=MAXC, d=1, num_idxs=SLAB)
                    gathers.append(g)
                    nc.sync.dma_start(out=emb_slab[112:114, :],
                                      in_=num_ones[:, s * SLAB:(s + 1) * SLAB])
                    # adj prefetch (u8 -> fp8 cast in DMA), contiguous rows
                    adj_tiles = []
                    for h in range(2):
                        at = adjp.tile([128, 7, N_CHAN], FP8, tag="adj_slab")
                        nc.gpsimd.dma_start(
                            out=at[:],
                            in_=adj_view[:, s * CPS + h * 7:s * CPS + (h + 1) * 7, :])
                        adj_tiles.append(at)
                    # transpose block: 14 chunks in 7 pairs -> fp16 device-major
                    dvs = []
                    for pr in range(CPS // 2):
                        tp = psT.tile([128, 2, D_AUG], F32, tag="tp")
                        for q in range(2):
                            j = pr * 2 + q
                            nc.tensor.transpose(
                                tp[:, q, :], emb_slab[:, j * 128:(j + 1) * 128],
                                id_t[:])
                        dv = dvp.tile([128, 2, D_AUG], F16, tag="dv")
                        nc.vector.tensor_copy(out=dv[:], in_=tp[:])
                        dvs.append(dv)
                    # aggregation matmul block (dense on PE)
                    for j in range(CPS):
                        gch = s * CPS + j
                        first = gch == 0
                        last = gch == NCHUNK - 1
                        dv = dvs[j // 2]
                        at = adj_tiles[j // 7]
                        nc.tensor.matmul(ps_nbr0[:], lhsT=dv[:, j % 2, :],
                                         rhs=at[:, j % 7, 0:500],
                                         start=first, stop=last)
                        nc.tensor.matmul(ps_nbr1[:], lhsT=dv[:, j % 2, :],
                                         rhs=at[:, j % 7, 500:1000],
                                         start=first, stop=last)
                    # device tower on this slab (fp16 weights/stream)
                    emb16 = slabp.tile([D_FEAT, SLAB], F16, tag="emb16")
                    nc.vector.tensor_copy(out=emb16[:], in_=emb_slab[0:D_FEAT, :])
                    for t0 in range(0, SLAB, 512):
                        t1 = min(t0 + 512, SLAB)
                        sl = slice(t0, t1)
                        ph1 = psH.tile([D1, 512], F32, tag="ph1")
                        nc.tensor.matmul(ph1[:, :t1 - t0], lhsT=w1_t[:],
                                         rhs=emb16[:, sl], start=True, stop=True)
                        h1s = h1p.tile([D1, 512], F16, tag="h1s")
                        nc.scalar.activation(h1s[:, :t1 - t0], ph1[:, :t1 - t0],
                                             Relu, bias=b1_t[:])
                        ph2 = psH.tile([D2P, 512], F32, tag="ph2")
                        nc.tensor.matmul(ph2[:, :t1 - t0], lhsT=w2_t[:],
                                         rhs=h1s[:, :t1 - t0], start=True, stop=True)
                        nc.scalar.activation(
                            h2full[:, s * SLAB + t0:s * SLAB + t1],
                            ph2[:, :t1 - t0], Relu, bias=b2_t[:])

                # evict aggregation partials
                nc.vector.tensor_copy(out=nbr_sb[:, 0:500], in_=ps_nbr0[:])
                nc.vector.tensor_copy(out=nbr_sb[:, 500:1000], in_=ps_nbr1[:])

            # ---------------- all-reduce partial neighbor sums ------------
            ar_in = dram.tile([D_AUG, N_CHAN], F32)
            ar_out = dram.tile([D_AUG, N_CHAN], F32)
            nc.gpsimd.dma_start(out=ar_in[:], in_=nbr_sb[:])
            nc.gpsimd.collective_compute(
                "AllReduce", mybir.AluOpType.add,
                replica_groups=[list(range(N_CORES))],
                ins=[ar_in[:].opt()], outs=[ar_out[:].opt()])
            nbr_red = big.tile([D_AUG, N_CHAN], F32)
            nc.sync.dma_start(out=nbr_red[:], in_=ar_out[:])

            # ---------------- channel tower (replicated, tiny) ------------
            with tc.tile_pool(name="chan", bufs=1) as chp, \
                 tc.tile_pool(name="psC", bufs=1, space="PSUM") as psC:
                deg_c = chp.tile([1, N_CHAN], F32)
                nc.sync.dma_start(out=deg_c[:], in_=nbr_red[113:114, :])
                nc.vector.tensor_scalar_max(deg_c[:], deg_c[:], 1.0)
                recip = chp.tile([1, N_CHAN], F32)
                nc.vector.reciprocal(recip[:], deg_c[:])
                ones_row = chp.tile([1, D_FEAT], F32)
                nc.vector.memset(ones_row[:], 1.0)
                nbr_mean = chp.tile([D_FEAT, N_CHAN], F32)
                for h in range(2):
                    sl = slice(h * 500, (h + 1) * 500)
                    pr = psC.tile([D_FEAT, 500], F32, tag="pr")
                    nc.tensor.matmul(pr[:], lhsT=ones_row[:], rhs=recip[:, sl],
                                     start=True, stop=True)
                    nc.vector.tensor_tensor(
                        out=nbr_mean[:, sl], in0=nbr_red[0:D_FEAT, sl],
                        in1=pr[:], op=mybir.AluOpType.mult)
                idg = chp.tile([16, 1024], F32)
                g = nc.gpsimd.ap_gather(
                    out_ap=idg[:], in_ap=cid_tbl_t[:], idxs_ap=cid_w_t[:],
                    channels=16, num_elems=N_CHAN, d=1, num_idxs=1024)
                gathers.append(g)
                msg_sb = chp.tile([D1, N_CHAN], F32)
                ch_sb = chp.tile([C1, N_CHAN], F32)
                fus_sb = chp.tile([FUS_OUT, N_CHAN], F32)
                for h in range(2):
                    sl = slice(h * 500, (h + 1) * 500)
                    pm = psC.tile([D1, 500], F32, tag="pm")
                    nc.tensor.matmul(pm[:], lhsT=msgw_t[:], rhs=nbr_mean[:, sl],
                                     start=True, stop=True)
                    nc.vector.tensor_scalar_add(msg_sb[:, sl], pm[:], msgb_t[:])
                    pc = psC.tile([C1, 500], F32, tag="pc")
                    nc.tensor.matmul(pc[:], lhsT=cwa_t[:], rhs=cnum_t[:, sl],
                                     start=True, stop=False)
                    nc.tensor.matmul(pc[:], lhsT=cwb_t[:], rhs=idg[:, sl],
                                     start=False, stop=True)
                    nc.vector.tensor_scalar_add(ch_sb[:, sl], pc[:], cb_t[:])
                for h in range(2):
                    sl = slice(h * 500, (h + 1) * 500)
                    pf = psC.tile([FUS_OUT, 500], F32, tag="pf")
                    nc.tensor.matmul(pf[:], lhsT=fwa_t[:], rhs=ch_sb[:, sl],
                                     start=True, stop=False)
                    nc.tensor.matmul(pf[:], lhsT=fwb_t[:], rhs=msg_sb[:, sl],
                                     start=False, stop=True)
                    nc.scalar.activation(fus_sb[:, sl], pf[:], Relu, bias=fb_t[:])
                    p2 = psC.tile([64, 500], F32, tag="p2")
                    nc.tensor.matmul(p2[:], lhsT=c1top_t[:], rhs=fus_sb[:, sl],
                                     start=True, stop=True)
                    nc.vector.tensor_copy(out=ch2full[:, sl], in_=p2[:])

            # ---------------- phase B: edge batch -------------------------
            with tc.tile_pool(name="bat", bufs=1) as bat, \
                 tc.tile_pool(name="gp", bufs=3) as gp, \
                 tc.tile_pool(name="z1p", bufs=2) as z1p, \
                 tc.tile_pool(name="psB", bufs=2, space="PSUM") as psB:
                hdev16 = bat.tile([64, NB], F16)
                hch16 = bat.tile([64, NB], F16)
                for blk in range(NB // BBLK):
                    ib = slice(blk * (BBLK // 16), (blk + 1) * (BBLK // 16))
                    ob = slice(blk * BBLK, (blk + 1) * BBLK)
                    hdf = gp.tile([64, BBLK], F32, tag="gtile")
                    g = nc.gpsimd.ap_gather(
                        out_ap=hdf[:], in_ap=h2full[:], idxs_ap=didx_t[:, ib],
                        channels=64, num_elems=NS, d=1, num_idxs=BBLK)
                    gathers.append(g)
                    nc.vector.tensor_copy(out=hdev16[:, ob], in_=hdf[:])
                    hcf = gp.tile([64, BBLK], F32, tag="gtile")
                    g = nc.gpsimd.ap_gather(
                        out_ap=hcf[:], in_ap=ch2full[:], idxs_ap=cidx_t[:, ib],
                        channels=64, num_elems=N_CHAN, d=1, num_idxs=BBLK)
                    gathers.append(g)
                    nc.vector.tensor_copy(out=hch16[:, ob], in_=hcf[:])
                for t in range(NB // 512):
                    sl = slice(t * 512, (t + 1) * 512)
                    pz1 = psB.tile([CC1, 512], F32, tag="pz1")
                    nc.tensor.matmul(pz1[:], lhsT=c1bot_t[:], rhs=hdev16[0:50, sl],
                                     start=True, stop=True)
                    nc.vector.tensor_tensor(out=pz1[:], in0=pz1[:],
                                            in1=hch16[0:CC1, sl],
                                            op=mybir.AluOpType.add)
                    z1 = z1p.tile([CC1, 512], F16, tag="z1")
                    nc.scalar.activation(z1[:], pz1[:], Relu, bias=cb1_t[:])
                    pz2 = psB.tile([CC2, 512], F32, tag="pz2")
                    nc.tensor.matmul(pz2[:], lhsT=c2_t[:], rhs=z1[:],
                                     start=True, stop=True)
                    z2 = z1p.tile([CC2, 512], F16, tag="z2")
                    nc.scalar.activation(z2[:], pz2[:], Relu, bias=cb2_t[:])
                    po = psB.tile([1, 512], F32, tag="po")
                    nc.tensor.matmul(po[:], lhsT=c3_t[:], rhs=z2[:],
                                     start=True, stop=True)
                    osl = z1p.tile([1, 512], F32, tag="osl")
                    nc.vector.tensor_scalar_add(osl[:], po[:], cb3_t[:])
                    nc.sync.dma_start(out=out_logits[:, sl], in_=osl[:])

            for g in gathers:
                add_dep_helper(g.ins, ll.ins,
                               reason="gather needs ap_gather library")

    nc.compile()
    return nc


def _wrap16(v):
    """[N] -> [16, N//16] with value for slot j at [j % 16, j // 16]."""
    return np.ascontiguousarray(v.reshape(-1, 16).T)


# gather position j <-> shard-local device d: d = (j % 128) * 196 + j // 128
_J = np.arange(NS)
_D_OF_J = (_J % 128) * NCHUNK + _J // 128        # device for position j
_J_OF_D = np.empty(NS, np.int64)
_J_OF_D[_D_OF_J] = _J                            # position for device d


def kernel(device_num, device_cat, channel_num, channel_id, adj, chan_idx,
           dev_idx, params):
    device_num = np.asarray(device_num, np.float32)
    device_cat = np.asarray(device_cat)
    channel_num = np.asarray(channel_num, np.float32)
    channel_id = np.asarray(channel_id)
    adj = np.asarray(adj)
    chan_idx = np.asarray(chan_idx)
    dev_idx = np.asarray(dev_idx)
    p = {k: np.asarray(v, np.float32) for k, v in params.items()}

    adj_u8 = adj.astype(np.uint8)
    shard = N_DEV // N_CORES  # 25000

    # ---- replicated host-side layout prep (params / index packing) ----
    emb_tbl = np.zeros((NT * 16, MAXC), np.float32)
    for k in range(NT):
        emb_tbl[16 * k:16 * k + 16, :CARDS[k]] = p[f"emb_{k}"].T
    perm = np.concatenate([1 + np.arange(112), [0]])  # feature permutation
    w1 = np.ascontiguousarray(p["dev_W1"][perm])
    msg_w = np.ascontiguousarray(p["msg_W"][perm])
    w2 = np.zeros((D1, D2P), np.float32)
    w2[:, :50] = p["dev_W2"]
    b2 = np.zeros((D2P, 1), np.float32)
    b2[:50, 0] = p["dev_b2"]
    ccw1_top = np.zeros((FUS_OUT, 64), np.float32)
    ccw1_top[:, :CC1] = p["cc_W1"][:FUS_OUT]
    cid = np.zeros(1024, channel_id.dtype)
    cid[:N_CHAN] = channel_id
    common = {
        "emb_tbl": emb_tbl,
        "ident": np.eye(D_AUG, dtype=np.float32),
        "w1": w1.astype(np.float16), "b1": p["dev_b1"].reshape(-1, 1),
        "w2": w2.astype(np.float16), "b2": b2,
        "msg_w": msg_w, "msg_b": p["msg_b"].reshape(-1, 1),
        "chan_w_a": np.ascontiguousarray(p["chan_W"][:8]),
        "chan_w_b": np.ascontiguousarray(p["chan_W"][8:]),
        "chan_b": p["chan_b"].reshape(-1, 1),
        "chan_num_t": np.ascontiguousarray(channel_num.T),
        "chan_id_tbl": np.ascontiguousarray(p["channel_id_emb"].T),
        "chan_id_w": _wrap16(cid.astype(np.int16)),
        "fus_wa": np.ascontiguousarray(p["fus_W"][:C1]),
        "fus_wb": np.ascontiguousarray(p["fus_W"][C1:]),
        "fus_b": p["fus_b"].reshape(-1, 1),
        "ccw1_top": ccw1_top,
        "ccw1_bot": np.ascontiguousarray(p["cc_W1"][FUS_OUT:]).astype(np.float16),
        "ccb1": p["cc_b1"].reshape(-1, 1),
        "ccw2": p["cc_W2"].astype(np.float16),
        "ccb2": p["cc_b2"].reshape(-1, 1),
        "ccw3": p["cc_W3"].astype(np.float16),
        "ccb3": p["cc_b3"].reshape(-1, 1),
    }

    # ---- shard device nodes + owner-shard the edge batch ----
    owner = dev_idx // shard
    in_maps, sels, counts = [], [], []
    for c in range(N_CORES):
        a = np.zeros((NS, N_CHAN), np.uint8)
        a[:shard] = adj_u8[c * shard:(c + 1) * shard]
        num_pad = np.zeros(NS, np.float32)
        num_pad[:shard] = device_num[c * shard:(c + 1) * shard, 0]
        no = np.ones((2, NS), np.float32)
        no[0] = num_pad[_D_OF_J]                 # position-ordered
        cat_pad = np.zeros((NS, NT), np.int16)
        cat_pad[:shard] = device_cat[c * shard:(c + 1) * shard]
        cat_pos = cat_pad[_D_OF_J]               # position-ordered
        cat_idx = np.concatenate([_wrap16(cat_pos[:, k]) for k in range(NT)], 0)

        sel = np.nonzero(owner == c)[0]
        cnt = sel.size
        assert cnt <= NB, f"core {c} batch {cnt} > {NB}"
        dloc = np.zeros(NB, np.int16)
        dloc[:cnt] = _J_OF_D[dev_idx[sel] - c * shard].astype(np.int16)
        cloc = np.zeros(NB, np.int16)
        cloc[:cnt] = chan_idx[sel].astype(np.int16)
        in_maps.append({
            "adj_s": a, "num_ones": no, "cat_idx": cat_idx,
            "dev_idx_w": np.tile(_wrap16(dloc), (4, 1)),
            "chan_idx_w": np.tile(_wrap16(cloc), (4, 1)),
            **common,
        })
        sels.append(sel)
        counts.append(cnt)

    if "nc" not in _CACHE:
        _CACHE["nc"] = _build_nc()
    nc = _CACHE["nc"]

    res = run_bass_kernel_spmd(nc, in_maps, core_ids=list(range(N_CORES)),
                               trace=TRACE, tmpdir=TRACE_DIR)
    _CACHE["last_result"] = res

    out = np.zeros(B, np.float32)
    for c in range(N_CORES):
        out[sels[c]] = res.results[c]["out_logits"][0, :counts[c]]
    return out
